# revision 61
# baseline (speedup 1.0000x reference)
"""GQA attention layer (dense_transformer) on 8 Trainium2 NeuronCores.

Sharding: data-parallel over batch (2) x tensor-parallel over head groups (4).
Core c handles batch c//4 and head-group c%4 (8 q heads, 2 kv heads).
Each core computes a partial output (its heads' contribution through its
Wo row-slice); the host sums the 4 partials per batch.

Per-core pipeline (all matmuls bf16, fp32 accumulation):
  P1 (fused QKV): per token tile: q AND k/v projections off one resident
      hs tile (single hst DMA pass, weight DMA in consumption order so the
      DMA-paced first tile streams against the wavefront); RMSNorm fused
      into the ScalarE squares via accum_out; RoPE (tables carry the
      128^-0.25 score scale and the norm weight); per-tile blocked XBAR
      DMA transposes -> qT/kT [d, it, head, i] (PE never transposes);
      v kept token-major with a ones column appended for the softmax
      denominators. The last tile's rms/rope is deferred into P2 (its raw
      psums staged to SBUF) so the P2 pool-transition barrier isn't
      gated on a dangling vector/scalar chain.
  P2 (fused attn + Wo): per 512-token block, per head: scoresT[j,i] =
      kT.T @ qT (diagonal tiles computed/exp'd only on the causally live
      columns), exp on ScalarE, triangle mask on DVE, PV with v_aug gives
      attn_out[i,d] + row sums in col 128; normalize by reciprocal row
      sums, PE-transpose -> aoT. Head loop is software-pipelined (scores
      of head h emitted before PV of head h-1 so the PE streams while
      ScalarE runs exp). Wo matmuls for the previous block interleave as
      PE filler between heads; wo prefetch is chunk-major/contiguous and
      pipelines against wo_tile consumption; outputs evict psum->SBUF
      alternating ScalarE/DVE, then DMA out via GpSimd SWDGE.
"""
import sys
from contextlib import ExitStack

import numpy as np

_REPO = "/opt/trn_rl_repo"
_PKGS = "/opt/pypackages"
for _p in (_REPO, _PKGS):
    if _p not in sys.path:
        sys.path.append(_p)

import ml_dtypes

BF16 = ml_dtypes.bfloat16

B, S, HIDDEN = 2, 2048, 4096
NUM_HEADS, NUM_KV_HEADS, HEAD_DIM = 32, 8, 128
EPS = 1e-6
ROPE_THETA = 10000.0
N_CORES = 8
TP = 4  # head groups
HQ = NUM_HEADS // TP        # 8 q heads per core
HKV = NUM_KV_HEADS // TP    # 2 kv heads per core
KT = HIDDEN // 128          # 32 k tiles
IT = S // 128               # 16 token tiles
IB = S // 512               # 4 token blocks (512 wide)


def _split_drain_waits():
    """walrus here rejects >1 sync wait on the tail Drain; split them."""
    from concourse import mybir
    from concourse.tile import TileContext
    from concourse.vector_clock import ScopedClock

    def _drain_and_barrier(self, tick_clock, wait_clock):
        drain_inst = self.nc.sync.drain()
        wait_clock.add_sem_waits(
            drain_inst.ins, ScopedClock({None: tick_clock.global_clock})
        )
        inst = drain_inst.ins
        si = inst.sync_info
        if si is not None and si.on_wait is not None and len(si.on_wait) > 1:
            waits = list(si.on_wait)
            del si.on_wait[1:]
            for i in range(1, len(waits)):
                e_inst = self.nc.sync.drain().ins
                if e_inst.sync_info is None:
                    e_inst.sync_info = mybir.SyncInfo(on_wait=[], on_update=[])
                e_inst.sync_info.on_wait.extend(waits[i : i + 1])
        self.nc.all_engine_barrier()
        assert self.sems is not None
        popped = self.nc._tile_sem_poison_stack.pop()
        assert popped is self._sem_poison
        self.nc.clear_and_free_semaphores(list(self.sems.allocated().values()))
        self.nc.all_engine_barrier()
        _fixup_wait_limits(self.nc)

    TileContext._drain_and_barrier = _drain_and_barrier


def _fixup_wait_limits(nc):
    """walrus in this image caps sync waits per instruction (DMA: hit at 3,
    Drain at 4+). Hoist excess waits onto nop instructions inserted just
    before the offender on the same engine (waits still complete before the
    original program point; engine order preserves semantics)."""
    from concourse import mybir

    def limit_for(inst):
        return 1

    def mk_nop(engine):
        bi = nc.engines[engine].nop(nofuse=True)
        inst = bi.ins if hasattr(bi, "ins") else bi
        for f in nc.m.functions:
            for blk in f.blocks:
                if blk.instructions and blk.instructions[-1] is inst:
                    blk.instructions.pop()
        return inst

    for f in nc.m.functions:
        for blk in f.blocks:
            out = []
            for inst in blk.instructions:
                si = inst.sync_info
                nw = len(si.on_wait) if si is not None and si.on_wait else 0
                lim = limit_for(inst)
                if nw > lim:
                    waits = list(si.on_wait)
                    del si.on_wait[lim:]
                    for w in waits[lim:]:
                        nop = mk_nop(inst.engine)
                        nop.sync_info = mybir.SyncInfo(on_wait=[w], on_update=[])
                        out.append(nop)
                out.append(inst)
            blk.instructions[:] = out


def build_bass():
    import concourse.bass as bass
    import concourse.tile as tile
    from concourse import mybir

    _split_drain_waits()

    f32 = mybir.dt.float32
    bf16 = mybir.dt.bfloat16
    AF = mybir.ActivationFunctionType
    ALU = mybir.AluOpType

    nc = bass.Bass("TRN2", target_bir_lowering=False, debug=False)

    hst = nc.dram_tensor("hst", [IT, 128, KT, 128], bf16, kind="ExternalInput")
    wq = nc.dram_tensor("wq", [128, KT, HQ * 128], bf16, kind="ExternalInput")
    wkv = nc.dram_tensor("wkv", [128, KT, 4 * 128], bf16, kind="ExternalInput")
    # chunk-major so each 512-col chunk is one fully-contiguous DMA (cheap
    # descriptor generation + pipelined arrival against wo_tile consumption)
    wo = nc.dram_tensor("wo", [8, 128, HQ, 512], bf16, kind="ExternalInput")
    cosq = nc.dram_tensor("cosq", [128, IT, 128], bf16, kind="ExternalInput")
    sinq = nc.dram_tensor("sinq", [128, IT, 128], bf16, kind="ExternalInput")
    cosk = nc.dram_tensor("cosk", [128, IT, 128], bf16, kind="ExternalInput")
    sink = nc.dram_tensor("sink", [128, IT, 128], bf16, kind="ExternalInput")
    masks = nc.dram_tensor("masks", [128, 4, 512], bf16, kind="ExternalInput")
    ident = nc.dram_tensor("ident", [128, 128], bf16, kind="ExternalInput")
    out = nc.dram_tensor("out", [S, HIDDEN], f32, kind="ExternalOutput")

    with tile.TileContext(nc) as tc, ExitStack() as top:
        const = top.enter_context(tc.tile_pool(name="const", bufs=1))
        res = top.enter_context(tc.tile_pool(name="res", bufs=1))

        # qT/kT are tile-major [d, it, head, i] so each rope tile's per-head
        # transposes land as ONE contiguous-destination XBAR DMA transpose
        # (frees the PE + DVE from transpose work entirely).
        qT = res.tile([128, IT, HQ, 128], bf16, tag="qT")
        kT = res.tile([128, IT, HKV, 128], bf16, tag="kT")
        v_aug = res.tile([128, HKV, IT, 129], bf16, tag="vaug")

        # ---------------- Phase 1: fused Q/K/V projection ----------------
        with ExitStack() as p1:
            wpool = p1.enter_context(tc.tile_pool(name="wqkv", bufs=1))
            tabp = p1.enter_context(tc.tile_pool(name="tabp", bufs=1))
            hpool = p1.enter_context(tc.tile_pool(name="hst", bufs=3))
            psq = p1.enter_context(tc.tile_pool(name="psq", bufs=2, space="PSUM"))
            stage = p1.enter_context(tc.tile_pool(name="stage", bufs=2))
            small = p1.enter_context(tc.tile_pool(name="small", bufs=4))

            wq_sb = wpool.tile([128, KT, HQ * 128], bf16, tag="wq")
            wkv_sb = wpool.tile([128, KT, 512], bf16, tag="wkv")

            # it=0 is DMA-paced (12.6 MB of weights stream in while its
            # matmuls consume them), and its matmuls run as three sequential
            # chains (kv, q-half0, q-half1). Emit weight chunks in exactly
            # that consumption order on the single in-order sync HWDGE queue,
            # with ht/tables/ident slotted in just before their first use;
            # masks are P2-only, so last.
            def wq_chunk(a, b, c0, c1):
                nc.sync.dma_start(
                    out=wq_sb[:, a:b, c0:c1], in_=wq.ap()[:, a:b, c0:c1]
                )

            def wkv_chunk(a, b):
                nc.sync.dma_start(out=wkv_sb[:, a:b, :], in_=wkv.ap()[:, a:b, :])

            ht_tiles = {}

            def ht_dma(i):
                t = hpool.tile([128, KT, 128], bf16, tag="ht", name=f"ht_{i}")
                nc.sync.dma_start(out=t, in_=hst.ap()[i])
                ht_tiles[i] = t

            ht_dma(0)
            wkv_chunk(0, 8)
            wkv_chunk(8, 16)
            wkv_chunk(16, 24)
            wkv_chunk(24, 32)
            wq_chunk(0, 8, 0, 512)
            wq_chunk(8, 16, 0, 512)
            ht_dma(1)
            wq_chunk(16, 24, 0, 512)
            wq_chunk(24, 32, 0, 512)
            wq_chunk(0, 8, 512, 1024)
            wq_chunk(8, 16, 512, 1024)
            ht_dma(2)
            wq_chunk(16, 24, 512, 1024)
            wq_chunk(24, 32, 512, 1024)
            # it=0's last matmul gates on the final weight chunk above, so
            # everything the PE doesn't consume directly (rope tables, masks)
            # arrives after: tile 0's rms chains have ~20us of psum-ring
            # slack before their table stall would back-pressure the PE.
            cos_sb = {}
            sin_sb = {}
            cos_sb["k"] = tabp.tile([128, IT, 128], bf16, tag="cosk", name="cosk_sb")
            sin_sb["k"] = tabp.tile([128, IT, 128], bf16, tag="sink", name="sink_sb")
            nc.sync.dma_start(out=cos_sb["k"], in_=cosk.ap())
            nc.sync.dma_start(out=sin_sb["k"], in_=sink.ap())
            cos_sb["q"] = tabp.tile([128, IT, 128], bf16, tag="cosq", name="cosq_sb")
            sin_sb["q"] = tabp.tile([128, IT, 128], bf16, tag="sinq", name="sinq_sb")
            nc.sync.dma_start(out=cos_sb["q"], in_=cosq.ap())
            nc.sync.dma_start(out=sin_sb["q"], in_=sinq.ap())
            masks_sb = const.tile([128, 4, 512], bf16, tag="masks")
            nc.sync.dma_start(out=masks_sb, in_=masks.ap())
            ident_sb = const.tile([128, 128], bf16, tag="ident")
            nc.sync.dma_start(out=ident_sb, in_=ident.ap())
            eps_sb = const.tile([128, 1], f32, tag="eps")
            nc.vector.memset(eps_sb, EPS)
            nc.vector.memset(v_aug[:, :, :, 128:129], 1.0)

            def rms_rope_group(psum_t, n_heads, which, h_base, it, dst):
                """psum_t: [128 i, n_heads*128] raw projections (PSUM).
                RMS-normalize each 128-wide head group (squares+sums fused on
                ScalarE via accum_out), apply RoPE (tables carry the
                128^-0.25 score scale and the norm weight), transpose each
                head to [d, i] and write dst[:, h_base+h, it*128:...]."""
                w = n_heads * 128
                qn = stage.tile(
                    [128, 512], f32, tag="qn", name=f"qn_{which}_{it}_{h_base}"
                )
                ss = small.tile(
                    [128, 4], f32, tag="ss", name=f"ss_{which}_{it}_{h_base}"
                )
                # squares' main output is dead (only accum_out matters): dump
                # it into qn, which the ts_mul below overwrites anyway.
                for h in range(n_heads):
                    nc.scalar.activation(
                        out=qn[:, h * 128 : (h + 1) * 128],
                        in_=psum_t[:, h * 128 : (h + 1) * 128],
                        func=AF.Square,
                        accum_out=ss[:, h : h + 1],
                    )
                rstd = small.tile(
                    [128, 4], f32, tag="rstd", name=f"rstd_{which}_{it}_{h_base}"
                )
                nc.scalar.activation(
                    out=rstd[:, 0:n_heads], in_=ss[:, 0:n_heads], func=AF.Sqrt,
                    scale=1.0 / HEAD_DIM, bias=eps_sb,
                )
                nc.vector.reciprocal(out=rstd[:, 0:n_heads], in_=rstd[:, 0:n_heads])
                for h in range(n_heads):
                    nc.vector.tensor_scalar_mul(
                        out=qn[:, h * 128 : (h + 1) * 128],
                        in0=psum_t[:, h * 128 : (h + 1) * 128],
                        scalar1=rstd[:, h : h + 1],
                    )
                qn3 = qn[:, 0:w].rearrange("p (h d) -> p h d", h=n_heads)
                cos_t = cos_sb[which][:, it, :]
                sin_t = sin_sb[which][:, it, :]
                ct = cos_t[:, 0:64][:, None, :].broadcast_to([128, n_heads, 64])
                cb = cos_t[:, 64:128][:, None, :].broadcast_to([128, n_heads, 64])
                st_ = sin_t[:, 0:64][:, None, :].broadcast_to([128, n_heads, 64])
                sb_ = sin_t[:, 64:128][:, None, :].broadcast_to([128, n_heads, 64])
                ta = stage.tile(
                    [128, 4, 64], f32, tag="ta", name=f"ta_{which}_{it}_{h_base}"
                )
                tb = stage.tile(
                    [128, 4, 64], f32, tag="tb", name=f"tb_{which}_{it}_{h_base}"
                )
                rq = stage.tile(
                    [128, 512], bf16, tag="rq", name=f"rq_{which}_{it}_{h_base}"
                )
                rq3 = rq[:, 0:w].rearrange("p (h d) -> p h d", h=n_heads)
                nc.vector.tensor_mul(out=ta[:, 0:n_heads], in0=qn3[:, :, 0:64], in1=ct)
                nc.vector.tensor_mul(out=tb[:, 0:n_heads], in0=qn3[:, :, 64:128], in1=st_)
                nc.vector.tensor_sub(
                    out=rq3[:, :, 0:64], in0=ta[:, 0:n_heads], in1=tb[:, 0:n_heads]
                )
                nc.vector.tensor_mul(out=ta[:, 0:n_heads], in0=qn3[:, :, 64:128], in1=cb)
                nc.vector.tensor_mul(out=tb[:, 0:n_heads], in0=qn3[:, :, 0:64], in1=sb_)
                nc.vector.tensor_add(
                    out=rq3[:, :, 64:128], in0=ta[:, 0:n_heads], in1=tb[:, 0:n_heads]
                )
                nc.sync.dma_start_transpose(
                    out=dst[:, it, h_base : h_base + n_heads, :],
                    in_=rq[:, 0:w],
                )

            # tile-15 rope-table slices, staged so the deferred chain can run
            # after the P1 table pool is gone (copied at it=0: off any
            # critical path)
            tabs15 = {}
            staged15 = {}

            for it in range(IT):
                if 1 <= it <= IT - 3:
                    ht_dma(it + 2)
                ht = ht_tiles[it]
                last = it == IT - 1
                ps_kv = psq.tile([128, 512], f32, tag="kv", name=f"kv_{it}")
                ps_q0 = psq.tile([128, 512], f32, tag="q0", name=f"q0_{it}")
                ps_q1 = psq.tile([128, 512], f32, tag="q1", name=f"q1_{it}")
                if it == 0:
                    # it=0 is DMA-paced: run kv, q-half0, q-half1 as three
                    # sequential chains so consumption tracks the weight DMA
                    # wavefront and each chain's rms starts while the next
                    # chain streams.
                    for kt in range(KT):
                        nc.tensor.matmul(ps_kv[:], ht[:, kt, :], wkv_sb[:, kt, :],
                                         start=kt == 0, stop=kt == KT - 1)
                    for kt in range(KT):
                        nc.tensor.matmul(ps_q0[:], ht[:, kt, :], wq_sb[:, kt, 0:512],
                                         start=kt == 0, stop=kt == KT - 1)
                    for kt in range(KT):
                        nc.tensor.matmul(ps_q1[:], ht[:, kt, :],
                                         wq_sb[:, kt, 512:1024],
                                         start=kt == 0, stop=kt == KT - 1)
                else:
                    # steady state: interleave the three chains per kt —
                    # consecutive matmuls hit different psum banks, which
                    # sustains a slightly higher PE rate than same-bank
                    # back-to-back accumulation.
                    for kt in range(KT):
                        st = kt == 0
                        sp = kt == KT - 1
                        nc.tensor.matmul(ps_kv[:], ht[:, kt, :], wkv_sb[:, kt, :],
                                         start=st, stop=sp)
                        nc.tensor.matmul(ps_q0[:], ht[:, kt, :], wq_sb[:, kt, 0:512],
                                         start=st, stop=sp)
                        nc.tensor.matmul(ps_q1[:], ht[:, kt, :],
                                         wq_sb[:, kt, 512:1024],
                                         start=st, stop=sp)
                for g in range(HKV):
                    sl = ps_kv[:, 256 + g * 128 : 256 + (g + 1) * 128]
                    nc.scalar.activation(
                        out=v_aug[:, g, it, 0:128], in_=sl, func=AF.Copy
                    )
                if it == 0:
                    for which in ("q", "k"):
                        for nm, src in (("cos", cos_sb[which]), ("sin", sin_sb[which])):
                            t15 = res.tile(
                                [128, 128], bf16, tag=f"t15_{nm}_{which}",
                                name=f"t15_{nm}_{which}"
                            )
                            nc.vector.tensor_copy(out=t15, in_=src[:, IT - 1, :])
                            tabs15[(nm, which)] = t15
                if not last:
                    rms_rope_group(ps_kv[:, 0:256], 2, "k", 0, it, kT)
                    rms_rope_group(ps_q0[:, :], 4, "q", 0, it, qT)
                    rms_rope_group(ps_q1[:, :], 4, "q", 4, it, qT)
                else:
                    # Last tile: every op left dangling here delays the P2
                    # pool-transition barrier (it waits on all P1 work), so
                    # just stage the raw psums to SBUF and run rms/rope/
                    # transposes inside P2 where they overlap attention.
                    stg_k = res.tile([128, 256], f32, tag="stg_k")
                    nc.scalar.activation(out=stg_k, in_=ps_kv[:, 0:256], func=AF.Copy)
                    staged15["k"] = stg_k
                    stg_q0 = res.tile([128, 512], f32, tag="stg_q0")
                    nc.scalar.activation(out=stg_q0, in_=ps_q0, func=AF.Copy)
                    staged15["q0"] = stg_q0
                    stg_q1 = res.tile([128, 512], f32, tag="stg_q1")
                    nc.vector.tensor_copy(out=stg_q1, in_=ps_q1)
                    staged15["q1"] = stg_q1

        # ---------- Phase 2: attention + output projection, interleaved ----------
        with ExitStack() as p2:
            wop = p2.enter_context(tc.tile_pool(name="wop", bufs=1))
            spsum = p2.enter_context(
                tc.tile_pool(name="spsum", bufs=3, space="PSUM")
            )
            ppsum = p2.enter_context(
                tc.tile_pool(name="ppsum", bufs=3, space="PSUM")
            )
            wpsum = p2.enter_context(
                tc.tile_pool(name="wpsum", bufs=2, space="PSUM")
            )
            ptpool = p2.enter_context(tc.tile_pool(name="pt", bufs=2))
            stage3 = p2.enter_context(tc.tile_pool(name="stage3", bufs=2))
            small3 = p2.enter_context(tc.tile_pool(name="small3", bufs=4))

            aoT = wop.tile([128, HQ, S], bf16, tag="aoT")
            wo_sb = wop.tile([128, 8, HQ, 512], bf16, tag="wo")
            for c in range(8):
                nc.sync.dma_start(out=wo_sb[:, c, :, :], in_=wo.ap()[c])

            # per-head score tiles of the in-flight head (and previous one)
            pts_by_head = {}
            ao_by_head = {}

            def scores(bi, t0, nt, h):
                g = h // (HQ // HKV)
                njt = t0 + nt
                wb = nt * 128
                pts = {}
                for jt in range(njt):
                    # block 0 runs before any wo_tile: borrow the idle wo
                    # psum slots to deepen the scores->exp pipeline while
                    # it is exp-paced.
                    stag = "wo" if (bi == 0 and jt % 2 == 1) else "ss"
                    ps_s = spsum.tile(
                        [128, 512], f32, tag=stag, name=f"ps_s_{bi}_{h}_{jt}"
                    ) if stag == "ss" else wpsum.tile(
                        [128, 512], f32, tag=stag, name=f"ps_s_{bi}_{h}_{jt}"
                    )
                    r0 = jt - t0 if jt >= t0 else 0
                    sc0 = r0 * 128
                    nc.tensor.matmul(
                        ps_s[:, sc0:wb],
                        kT[:, jt, g, :],
                        qT[:, t0 + r0 : t0 + nt, h, :],
                        start=True,
                        stop=True,
                    )
                    pt_t = ptpool.tile(
                        [128, 512], bf16, tag=f"pt{jt}", name=f"pt_{bi}_{h}_{jt}"
                    )
                    if jt >= t0:
                        # diagonal tile at offset r: PV never reads columns
                        # below r*128, so exp only the live region and mask
                        # only the triangle column block.
                        c0 = r0 * 128
                        nc.scalar.activation(
                            out=pt_t[:, c0:wb], in_=ps_s[:, c0:wb], func=AF.Exp
                        )
                        nc.vector.tensor_mul(
                            out=pt_t[:, c0 : c0 + 128],
                            in0=pt_t[:, c0 : c0 + 128],
                            in1=masks_sb[:, r0, c0 : c0 + 128],
                        )
                    else:
                        nc.scalar.activation(
                            out=pt_t[:, 0:wb], in_=ps_s[:, 0:wb], func=AF.Exp
                        )
                    pts[jt] = pt_t
                pts_by_head[h] = pts

            def pv(bi, t0, nt, h):
                g = h // (HQ // HKV)
                pts = pts_by_head[h]
                ao_blk = stage3.tile(
                    [128, 4, 128], bf16, tag="ao", name=f"ao_{bi}_{h}"
                )
                for itl in range(nt):
                    it_g = t0 + itl
                    po = ppsum.tile(
                        [128, 129], f32, tag="pp", name=f"po_{bi}_{h}_{itl}"
                    )
                    for jt in range(it_g + 1):
                        nc.tensor.matmul(
                            po[:],
                            pts[jt][:, itl * 128 : (itl + 1) * 128],
                            v_aug[:, g, jt, :],
                            start=(jt == 0),
                            stop=(jt == it_g),
                        )
                    rec = small3.tile(
                        [128, 1], f32, tag="rec", name=f"rec_{bi}_{h}_{itl}"
                    )
                    nc.vector.reciprocal(out=rec, in_=po[:, 128:129])
                    nc.vector.tensor_scalar_mul(
                        out=ao_blk[:, itl, :], in0=po[:, 0:128], scalar1=rec
                    )
                ao_by_head[h] = ao_blk

            def transposes(bi, t0, nt, h):
                # PE transposes here: the sync DMA queue is saturated with the
                # wo prefetch in exactly this window, and aoT feeds wo_tile
                # soon after — XBAR transposes would queue behind the load.
                ao_blk = ao_by_head[h]
                for itl in range(nt):
                    it_g = t0 + itl
                    ps_t = ppsum.tile(
                        [128, 128], bf16, tag="pp", name=f"tp2_{bi}_{h}_{itl}"
                    )
                    nc.tensor.transpose(ps_t, ao_blk[:, itl, :], ident_sb)
                    nc.vector.tensor_copy(
                        out=aoT[:, h, it_g * 128 : (it_g + 1) * 128], in_=ps_t
                    )

            def wo_tile(m):
                for c in range(8):
                    wo_ps = wpsum.tile(
                        [128, 512], f32, tag="wo", name=f"wo_{m}_{c}"
                    )
                    for k in range(HQ):
                        nc.tensor.matmul(
                            wo_ps[:],
                            aoT[:, k, m * 128 : (m + 1) * 128],
                            wo_sb[:, c, k, :],
                            start=(k == 0),
                            stop=(k == HQ - 1),
                        )
                    ost = stage3.tile(
                        [128, 512], f32, tag="ost", name=f"ost_{m}_{c}", bufs=3
                    )
                    # split evicts between ScalarE and DVE: 8 on one engine
                    # would sit between attention exps (or masks) in that
                    # queue and push the exp-paced pipeline back ~5us per
                    # wo_tile; alternating halves the intrusion on each.
                    if c % 2 == 0:
                        nc.scalar.activation(out=ost, in_=wo_ps, func=AF.Copy)
                    else:
                        nc.vector.tensor_copy(out=ost, in_=wo_ps)
                    nc.gpsimd.dma_start(
                        out=out.ap()[m * 128 : (m + 1) * 128, c * 512 : (c + 1) * 512],
                        in_=ost,
                    )

            def rms_rope_deferred(src, n_heads, which, h_base, dst):
                """Tile-15's rms/rope/transposes, run inside P2 off the staged
                SBUF copy of its projection psums."""
                it = IT - 1
                w = n_heads * 128
                qn = stage3.tile(
                    [128, 512], f32, tag="qn15", name=f"qn15_{which}_{h_base}"
                )
                ss = small3.tile(
                    [128, 4], f32, tag="ss15", name=f"ss15_{which}_{h_base}"
                )
                for h in range(n_heads):
                    nc.scalar.activation(
                        out=qn[:, h * 128 : (h + 1) * 128],
                        in_=src[:, h * 128 : (h + 1) * 128],
                        func=AF.Square,
                        accum_out=ss[:, h : h + 1],
                    )
                rstd = small3.tile(
                    [128, 4], f32, tag="rstd15", name=f"rstd15_{which}_{h_base}"
                )
                nc.scalar.activation(
                    out=rstd[:, 0:n_heads], in_=ss[:, 0:n_heads], func=AF.Sqrt,
                    scale=1.0 / HEAD_DIM, bias=eps_sb,
                )
                nc.vector.reciprocal(out=rstd[:, 0:n_heads], in_=rstd[:, 0:n_heads])
                for h in range(n_heads):
                    nc.vector.tensor_scalar_mul(
                        out=qn[:, h * 128 : (h + 1) * 128],
                        in0=src[:, h * 128 : (h + 1) * 128],
                        scalar1=rstd[:, h : h + 1],
                    )
                qn3 = qn[:, 0:w].rearrange("p (h d) -> p h d", h=n_heads)
                cos_t = tabs15[("cos", which)]
                sin_t = tabs15[("sin", which)]
                ct = cos_t[:, 0:64][:, None, :].broadcast_to([128, n_heads, 64])
                cb = cos_t[:, 64:128][:, None, :].broadcast_to([128, n_heads, 64])
                st_ = sin_t[:, 0:64][:, None, :].broadcast_to([128, n_heads, 64])
                sb_ = sin_t[:, 64:128][:, None, :].broadcast_to([128, n_heads, 64])
                ta = stage3.tile(
                    [128, 4, 64], f32, tag="ta15", name=f"ta15_{which}_{h_base}"
                )
                tb = stage3.tile(
                    [128, 4, 64], f32, tag="tb15", name=f"tb15_{which}_{h_base}"
                )
                # per-group tag: each rq must stay live until its transposes,
                # which are emitted later; a shared ring would create a
                # forward WAR dependency onto not-yet-queued PE work.
                rq = stage3.tile(
                    [128, 512], bf16, tag=f"rq15_{which}_{h_base}", bufs=1,
                    name=f"rq15_{which}_{h_base}"
                )
                rq3 = rq[:, 0:w].rearrange("p (h d) -> p h d", h=n_heads)
                nc.vector.tensor_mul(out=ta[:, 0:n_heads], in0=qn3[:, :, 0:64], in1=ct)
                nc.vector.tensor_mul(out=tb[:, 0:n_heads], in0=qn3[:, :, 64:128], in1=st_)
                nc.vector.tensor_sub(
                    out=rq3[:, :, 0:64], in0=ta[:, 0:n_heads], in1=tb[:, 0:n_heads]
                )
                nc.vector.tensor_mul(out=ta[:, 0:n_heads], in0=qn3[:, :, 64:128], in1=cb)
                nc.vector.tensor_mul(out=tb[:, 0:n_heads], in0=qn3[:, :, 0:64], in1=sb_)
                nc.vector.tensor_add(
                    out=rq3[:, :, 64:128], in0=ta[:, 0:n_heads], in1=tb[:, 0:n_heads]
                )
                nc.sync.dma_start_transpose(
                    out=dst[:, it, h_base : h_base + n_heads, :],
                    in_=rq[:, 0:w],
                )

            # NOTE: splitting the last block (256+128+128) to shrink the
            # pure-Wo tail was tried and is a net loss: exp cost scales with
            # score-tile COUNT (njt/head), not width, so sub-blocks of the
            # last block pay ~2x scalar+matmul instruction overhead (+32us
            # tensor, +39us scalar for -41us tail).
            blocks = [(0, 4), (4, 4), (8, 4), (12, 4)]
            wo_ready = []
            for bi, (t0, nt) in enumerate(blocks):
                for h in range(HQ):
                    scores(bi, t0, nt, h)
                    if h >= 1:
                        pv(bi, t0, nt, h - 1)
                        transposes(bi, t0, nt, h - 1)
                    if h % 2 == 1 and wo_ready:
                        wo_tile(wo_ready.pop(0))
                    if bi == 1 and h == 1:
                        # tile-15 projections epilogue: its vector/scalar
                        # chain drains while the PE streams wo_tile(0)
                        rms_rope_deferred(staged15["q0"], 4, "q", 0, qT)
                        rms_rope_deferred(staged15["q1"], 4, "q", 4, qT)
                        rms_rope_deferred(staged15["k"], 2, "k", 0, kT)
                pv(bi, t0, nt, HQ - 1)
                transposes(bi, t0, nt, HQ - 1)
                wo_ready.extend(range(t0, t0 + nt))
            for m in wo_ready:
                wo_tile(m)

    return nc


def prep_core_inputs(hidden_states, position_ids, Wq, Wk, Wv, Wo, q_norm_w, k_norm_w):
    """Host-side shard + layout prep. Returns list of 8 in_maps."""
    pos = np.asarray(position_ids).reshape(-1).astype(np.float64)  # [S]
    inv_freq = 1.0 / (
        ROPE_THETA ** (np.arange(0, HEAD_DIM, 2, dtype=np.float64) / HEAD_DIM)
    )  # [64]
    ang = pos[:, None] * inv_freq[None, :]  # [S, 64]
    emb = np.concatenate([ang, ang], axis=1)  # [S, 128]
    scale = HEAD_DIM ** (-0.25)
    cos = (np.cos(emb) * scale).astype(np.float32)  # [S, 128]
    sin = (np.sin(emb) * scale).astype(np.float32)
    qw = np.asarray(q_norm_w, dtype=np.float32)
    kw = np.asarray(k_norm_w, dtype=np.float32)
    qw_roll = np.concatenate([qw[64:], qw[:64]])
    kw_roll = np.concatenate([kw[64:], kw[:64]])

    def table(t):  # [S,128] -> [128, IT, 128]
        return np.ascontiguousarray(
            t.reshape(IT, 128, 128).transpose(1, 0, 2)
        )

    cosq_t = table(cos * qw[None, :]).astype(BF16)
    sinq_t = table(sin * qw_roll[None, :]).astype(BF16)
    cosk_t = table(cos * kw[None, :]).astype(BF16)
    sink_t = table(sin * kw_roll[None, :]).astype(BF16)

    # causal masks for the 4 diagonal offsets
    jj = np.arange(128)[:, None]
    ii = np.arange(512)[None, :]
    masks = np.stack(
        [(jj <= ii - 128 * r).astype(np.float32) for r in range(4)]
    ).transpose(1, 0, 2)  # [128, 4, 512]
    masks = masks.astype(BF16)
    ident = np.eye(128, dtype=np.float32).astype(BF16)

    hs = np.asarray(hidden_states, dtype=np.float32)
    Wq = np.asarray(Wq, dtype=np.float32)
    Wk = np.asarray(Wk, dtype=np.float32)
    Wv = np.asarray(Wv, dtype=np.float32)
    Wo = np.asarray(Wo, dtype=np.float32)

    hst_b = []
    for b in range(B):
        hsT = hs[b].T.astype(BF16)  # [4096, 2048]
        # -> [IT, 128(i), KT, 128(k)]: hst[it, ip, kt, kp] = hsT[kt*128+kp, it*128+ip]
        t = hsT.reshape(KT, 128, IT, 128).transpose(2, 1, 0, 3)
        hst_b.append(np.ascontiguousarray(t))

    in_maps = []
    for c in range(N_CORES):
        b, grp = divmod(c, TP)
        wq_s = Wq[:, grp * HQ * 128 : (grp + 1) * HQ * 128].astype(BF16)
        wq_t = np.ascontiguousarray(
            wq_s.reshape(KT, 128, HQ * 128).transpose(1, 0, 2)
        )  # [128, KT, 1024]
        wk_s = Wk[:, grp * HKV * 128 : (grp + 1) * HKV * 128]
        wv_s = Wv[:, grp * HKV * 128 : (grp + 1) * HKV * 128]
        wkv_s = np.concatenate([wk_s, wv_s], axis=1).astype(BF16)  # [4096, 512]
        wkv_t = np.ascontiguousarray(
            wkv_s.reshape(KT, 128, 512).transpose(1, 0, 2)
        )  # [128, KT, 512]
        wo_s = Wo[grp * HQ * 128 : (grp + 1) * HQ * 128, :].astype(BF16)  # [1024, 4096]
        wo_t = np.ascontiguousarray(
            wo_s.reshape(HQ, 128, 8, 512).transpose(2, 1, 0, 3)
        )  # [8, 128, HQ, 512] chunk-major
        in_maps.append(
            {
                "hst": hst_b[b],
                "wq": wq_t,
                "wkv": wkv_t,
                "wo": wo_t,
                "cosq": cosq_t,
                "sinq": sinq_t,
                "cosk": cosk_t,
                "sink": sink_t,
                "masks": masks,
                "ident": ident,
            }
        )
    return in_maps


def kernel(hidden_states, position_ids, Wq, Wk, Wv, Wo, q_norm_w, k_norm_w,
           _trace=False, _tmpdir=None):
    from concourse.bass_utils import run_bass_kernel_spmd

    nc = build_bass()
    in_maps = prep_core_inputs(
        hidden_states, position_ids, Wq, Wk, Wv, Wo, q_norm_w, k_norm_w
    )
    kwargs = {}
    if _trace:
        kwargs = dict(trace=True, tmpdir=_tmpdir)
    res = run_bass_kernel_spmd(nc, in_maps, list(range(N_CORES)), **kwargs)
    partials = [res.results[c]["out"] for c in range(N_CORES)]
    outb = [
        np.sum([partials[b * TP + g] for g in range(TP)], axis=0, dtype=np.float32)
        for b in range(B)
    ]
    full = np.stack(outb).astype(np.float32)  # [2, 2048, 4096]
    if _trace:
        kernel._last_result = res
    return full
